# revision 1
# baseline (speedup 1.0000x reference)
"""Multi-head self-attention (B=2,S=2048,E=2048,H=16) on 8 trn2 NeuronCores.

Sharding: tensor-parallel over heads. Each core owns 2 heads (256 channels):
  - computes q/k/v projections for its heads only (column-sharded Wq/Wk/Wv)
  - runs causal attention for its (batch, head) pairs
  - computes a partial output projection (row-sharded Wo)
Host sums the 8 partial outputs (the all-reduce of the TP scheme).

Device layouts (per core):
  xT   [E, B*S]    x transposed; rhs for q/k projections, lhsT for v proj
  qT,kT [D, S]     per (b,h); head dim on partitions
  v    [S, C]      natural layout; lhsT of the ctx matmul
  scores sT [k,q]  transposed scores = kT.T @ qT tiles
  softmax: exp without max-subtraction (inputs are well-scaled); denominator
  via ones-vector matmul over the accumulated exp-sum, reciprocal,
  gpsimd partition_broadcast, fused into the ctx PSUM drain.
  ctxT [D, S]      per (b,h); directly the lhsT of the Wo matmul.

All matmuls run in float32r (TF32-class, 4x faster than true fp32 on PE).
"""
import sys

sys.path.insert(0, "/opt/trn_rl_repo")
import numpy as np

B, S, E, H = 2, 2048, 2048, 16
D = 128
NCORES = 8
HL = H // NCORES      # heads per core
C = HL * D            # channels per core
BS = B * S
SB = 512              # s-block (projection) / q-block (attention) width
NSB = S // SB         # 4 s-blocks per batch
NET = E // 128        # 16 contraction tiles
NST = S // 128        # 16 s-subtiles per batch

_CACHE = {}


def _build_nc(kloop=None, phases=("proj", "attn", "wo")):
    import concourse.mybir as mybir
    import concourse.tile as tile
    from concourse import bacc

    F32 = mybir.dt.float32
    F32R = mybir.dt.float32r
    AF = mybir.ActivationFunctionType
    OP = mybir.AluOpType
    SCALE = 1.0 / float(np.sqrt(D))

    nc = bacc.Bacc(None, target_bir_lowering=False)

    xT_d = nc.dram_tensor("xT", [E, BS], F32R, kind="ExternalInput")
    wq_d = nc.dram_tensor("wq", [E, C], F32R, kind="ExternalInput")
    wk_d = nc.dram_tensor("wk", [E, C], F32R, kind="ExternalInput")
    wv_d = nc.dram_tensor("wv", [E, C], F32R, kind="ExternalInput")
    wo_d = nc.dram_tensor("wo", [C, E], F32R, kind="ExternalInput")
    bq_d = nc.dram_tensor("bq", [HL, D], F32, kind="ExternalInput")
    bk_d = nc.dram_tensor("bk", [HL, D], F32, kind="ExternalInput")
    bv_d = nc.dram_tensor("bv", [HL, D], F32, kind="ExternalInput")
    mk_d = nc.dram_tensor("mk", [128, 128], F32R, kind="ExternalInput")
    out_d = nc.dram_tensor("out", [BS, E], F32, kind="ExternalOutput")
    import os
    DBG = bool(os.environ.get("KDBG"))
    if DBG:
        dbg_q = nc.dram_tensor("dbg_q", [128, HL, S], F32, kind="ExternalOutput")
        dbg_k = nc.dram_tensor("dbg_k", [128, HL, S], F32, kind="ExternalOutput")
        dbg_v = nc.dram_tensor("dbg_v", [128, NST, C], F32, kind="ExternalOutput")
        dbg_c = nc.dram_tensor("dbg_c", [128, HL, S], F32, kind="ExternalOutput")

    with tile.TileContext(nc) as tc:
        with (
            tc.tile_pool(name="const", bufs=1) as cp,
            tc.tile_pool(name="big", bufs=1) as bigp,
            tc.tile_pool(name="xt", bufs=2) as xtp,
            tc.tile_pool(name="pp", bufs=6) as ppool,
            tc.tile_pool(name="work", bufs=2) as wp,
            tc.tile_pool(name="osb", bufs=3) as osp,
        ):
            # ---- constants / weights resident in SBUF ----
            wq_t = cp.tile([128, NET, C], F32R)
            wk_t = cp.tile([128, NET, C], F32R)
            wv_t = cp.tile([128, NET, C], F32R)
            wo_t = cp.tile([128, HL, E], F32R)
            wq_r = wq_d.rearrange("(eo p) c -> p eo c", p=128)
            wk_r = wk_d.rearrange("(eo p) c -> p eo c", p=128)
            wv_r = wv_d.rearrange("(eo p) c -> p eo c", p=128)
            for w_t, w_r in ((wq_t, wq_r), (wk_t, wk_r), (wv_t, wv_r)):
                nc.sync.dma_start(w_t[:, :1, :], w_r[:, :1, :])
            for w_t, w_r in ((wq_t, wq_r), (wk_t, wk_r), (wv_t, wv_r)):
                nc.scalar.dma_start(w_t[:, 1:4, :], w_r[:, 1:4, :])
            for w_t, w_r in ((wq_t, wq_r), (wk_t, wk_r), (wv_t, wv_r)):
                nc.scalar.dma_start(w_t[:, 4:, :], w_r[:, 4:, :])

            mk_t = cp.tile([128, 128], F32R)

            bq_t = cp.tile([128, HL], F32)
            bk_t = cp.tile([128, HL], F32)
            bv_t = cp.tile([128, HL], F32)
            for h in range(HL):
                nc.scalar.dma_start(bq_t[:, h : h + 1], bq_d[h, :, None])
                nc.scalar.dma_start(bk_t[:, h : h + 1], bk_d[h, :, None])
                nc.scalar.dma_start(bv_t[:, h : h + 1], bv_d[h, :, None])

            ones_f = cp.tile([128, 1], F32)
            nc.vector.memset(ones_f[:], 1.0)
            ones_c = cp.tile([128, 1], F32R)
            nc.vector.tensor_copy(ones_c[:], ones_f[:])

            import contextlib
            if kloop is not None:
                # timing build: preload late consts, then repeat body kloop times
                nc.sync.dma_start(mk_t[:], mk_d[:])
                nc.sync.dma_start(wo_t[:], wo_d.rearrange("(co p) e -> p co e", p=128))
                loop_cm = tc.For_i(0, kloop, 1)
            else:
                loop_cm = contextlib.nullcontext()
            with loop_cm:
              for b in range(B):
                  # per-batch activations (bufs=1 -> reused across b)
                  qT = bigp.tile([128, HL, S], F32R, tag="qT")
                  kT = bigp.tile([128, HL, S], F32R, tag="kT")
                  v_t = bigp.tile([128, NST, C], F32R, tag="v")
                  cxT = bigp.tile([128, HL, S], F32R, tag="cxT")

                  # ---------- projections ----------
                  with tc.tile_pool(name="ps_proj", bufs=1, space="PSUM") as pp:
                      for sb in range(NSB):
                          s0 = sb * SB
                          qps = [pp.tile([128, SB], F32, tag=f"q{h}", name=f"qps{h}") for h in range(HL)]
                          kps = [pp.tile([128, SB], F32, tag=f"k{h}", name=f"kps{h}") for h in range(HL)]
                          vps = [pp.tile([128, C], F32, tag=f"v{j}", name=f"vps{j}") for j in range(4)]
                          for eg in range(NET // 4):
                              xt = xtp.tile([128, 4, SB], F32R, tag="xt")
                              nc.sync.dma_start(
                                  xt[:],
                                  xT_d.rearrange("(eo p) s -> p eo s", p=128)[
                                      :, eg * 4 : (eg + 1) * 4, b * S + s0 : b * S + s0 + SB
                                  ],
                              )
                              for ei in range(4):
                                  et = eg * 4 + ei
                                  st_flags = dict(start=(et == 0), stop=(et == NET - 1))
                                  for h in range(HL):
                                      hs = slice(h * D, (h + 1) * D)
                                      nc.tensor.matmul(qps[h][:], wq_t[:, et, hs], xt[:, ei, :], **st_flags)
                                      nc.tensor.matmul(kps[h][:], wk_t[:, et, hs], xt[:, ei, :], **st_flags)
                                  for st in range(4):
                                      nc.tensor.matmul(
                                          vps[st][:],
                                          xt[:, ei, st * 128 : (st + 1) * 128],
                                          wv_t[:, et, :],
                                          **st_flags,
                                      )
                          for h in range(HL):
                              nc.scalar.activation(
                                  qT[:, h, s0 : s0 + SB], qps[h][:], AF.Identity,
                                  bias=bq_t[:, h : h + 1],
                              )
                              nc.scalar.activation(
                                  kT[:, h, s0 : s0 + SB], kps[h][:], AF.Identity,
                                  bias=bk_t[:, h : h + 1],
                              )
                          for st in range(4):
                              nc.vector.tensor_copy(v_t[:, sb * 4 + st, :], vps[st][:])

                  if b == 0 and kloop is None:
                      # late const loads: needed only from attention / wo phases on
                      nc.scalar.dma_start(mk_t[:], mk_d[:])
                      nc.scalar.dma_start(wo_t[:], wo_d.rearrange("(co p) e -> p co e", p=128))

                  # ---------- causal attention ----------
                  if "attn" not in phases:
                      continue
                  with (
                      tc.tile_pool(name="ps_s", bufs=4, space="PSUM") as pss,
                      tc.tile_pool(name="ps_ctx", bufs=2, space="PSUM") as psc,
                      tc.tile_pool(name="ps_den", bufs=2, space="PSUM") as psd,
                  ):
                      for h in range(HL):
                          hs = slice(h * D, (h + 1) * D)
                          for qb in range(NSB):
                              q0 = qb * SB
                              nkt = (qb + 1) * (SB // 128)
                              ctxps = psc.tile([128, SB], F32, tag="ctx")
                              dps = psd.tile([1, SB], F32, tag="den")
                              for kt in range(nkt):
                                  j = kt - (nkt - SB // 128)
                                  off = 128 * j if j > 0 else 0  # fully-masked cols skipped
                                  w = SB - off
                                  sps = pss.tile([128, SB], F32, tag="s")
                                  nc.tensor.matmul(
                                      sps[:, :w],
                                      kT[:, h, kt * 128 : (kt + 1) * 128],
                                      qT[:, h, q0 + off : q0 + SB],
                                      start=True, stop=True,
                                  )
                                  p = ppool.tile([128, SB], F32R, tag="p")
                                  nc.scalar.activation(p[:, :w], sps[:, :w], AF.Exp, scale=SCALE)
                                  if j >= 0:
                                      # triangle block = first 128 live columns
                                      nc.vector.tensor_tensor(
                                          p[:, :128], p[:, :128], mk_t[:], OP.mult
                                      )
                                  nc.tensor.matmul(
                                      ctxps[:, off:SB], v_t[:, kt, hs], p[:, :w],
                                      start=(kt == 0), stop=(kt == nkt - 1),
                                  )
                                  nc.tensor.matmul(
                                      dps[:, off:SB], ones_c[:], p[:, :w],
                                      start=(kt == 0), stop=(kt == nkt - 1),
                                  )
                              rec = wp.tile([1, SB], F32, tag="rec")
                              nc.vector.reciprocal(rec[:], dps[:])
                              bt = wp.tile([128, SB], F32, tag="B")
                              nc.gpsimd.partition_broadcast(bt[:], rec[:])
                              nc.vector.tensor_tensor(
                                  cxT[:, h, q0 : q0 + SB], ctxps[:], bt[:], OP.mult
                              )
                              nc.scalar.activation(
                                  cxT[:, h, q0 : q0 + SB], cxT[:, h, q0 : q0 + SB],
                                  AF.Identity, bias=bv_t[:, h : h + 1],
                              )

                  if DBG and b == 0:
                      nc.sync.dma_start(dbg_q[:], qT[:].bitcast(F32))
                      nc.sync.dma_start(dbg_k[:], kT[:].bitcast(F32))
                      nc.sync.dma_start(dbg_v[:], v_t[:].bitcast(F32))
                      nc.sync.dma_start(dbg_c[:], cxT[:].bitcast(F32))

                  # ---------- output projection (partial; host sums cores) ----------
                  if "wo" not in phases:
                      continue
                  with tc.tile_pool(name="ps_o", bufs=4, space="PSUM") as pso:
                      for qt in range(NST):
                          osb = osp.tile([128, E], F32, tag="osb")
                          for eb in range(E // SB):
                              ops = pso.tile([128, SB], F32, tag="o", name="ops")
                              for h in range(HL):
                                  nc.tensor.matmul(
                                      ops[:],
                                      cxT[:, h, qt * 128 : (qt + 1) * 128],
                                      wo_t[:, h, eb * SB : (eb + 1) * SB],
                                      start=(h == 0), stop=(h == HL - 1),
                                  )
                              dst = osb[:, eb * SB : (eb + 1) * SB]
                              if eb % 2 == 0:
                                  nc.scalar.copy(dst, ops[:])
                              else:
                                  nc.vector.tensor_copy(dst, ops[:])
                          dma_eng = nc.sync if qt % 2 == 0 else nc.gpsimd
                          dma_eng.dma_start(
                              out_d[b * S + qt * 128 : b * S + (qt + 1) * 128, :], osb[:]
                          )

    nc.compile()
    return nc


def make_in_maps(x, Wq, bq, Wk, bk, Wv, bv, Wo, bo):
    xT = np.ascontiguousarray(np.asarray(x, np.float32).reshape(BS, E).T)
    ki = np.arange(128)[:, None]
    qi = np.arange(128)[None, :]
    masks = (ki <= qi).astype(np.float32)
    in_maps = []
    for c in range(NCORES):
        ch = slice(c * C, (c + 1) * C)
        in_maps.append(
            {
                "xT": xT,
                "wq": np.ascontiguousarray(np.asarray(Wq, np.float32)[ch, :].T),
                "wk": np.ascontiguousarray(np.asarray(Wk, np.float32)[ch, :].T),
                "wv": np.ascontiguousarray(np.asarray(Wv, np.float32)[ch, :].T),
                "wo": np.ascontiguousarray(np.asarray(Wo, np.float32)[:, ch].T),
                "bq": np.asarray(bq, np.float32)[ch].reshape(HL, D),
                "bk": np.asarray(bk, np.float32)[ch].reshape(HL, D),
                "bv": np.asarray(bv, np.float32)[ch].reshape(HL, D),
                "mk": masks,
            }
        )
    return in_maps


def get_nc(kloop=None, phases=("proj", "attn", "wo")):
    key = ("nc", kloop, phases)
    if key not in _CACHE:
        _CACHE[key] = _build_nc(kloop, phases)
    return _CACHE[key]


def kernel(x, Wq, bq, Wk, bk, Wv, bv, Wo, bo):
    from concourse.bass_utils import run_bass_kernel_spmd

    nc = get_nc()
    in_maps = make_in_maps(x, Wq, bq, Wk, bk, Wv, bv, Wo, bo)
    res = run_bass_kernel_spmd(nc, in_maps, core_ids=list(range(NCORES)))
    acc = np.zeros((BS, E), np.float64)
    for r in res.results:
        acc += r["out"].astype(np.float64)
    acc += np.asarray(bo, np.float64)[None, :]
    return acc.astype(np.float32).reshape(B, S, E)


if __name__ == "__main__":
    import reference

    inputs = {k: np.asarray(v) for k, v in reference.setup_inputs().items()}
    expected = np.asarray(reference.reference(**inputs))
    actual = kernel(**inputs)
    err = np.linalg.norm(actual - expected) / np.linalg.norm(expected)
    print("Relative error:", err)

def _debug_dump():
    import reference
    inputs = {k: np.asarray(v) for k, v in reference.setup_inputs().items()}
    expected = np.asarray(reference.reference(**inputs))
    actual = kernel(**inputs)
    np.save("/tmp/actual.npy", actual)
    np.save("/tmp/expected.npy", expected)
    print("saved")

